# revision 9
# baseline (speedup 1.0000x reference)
"""BiLSTM-CRF Trainium2 kernel.

Strategy (data-parallel over batch, 8 sentences per core on 8 cores):
  P1: embedding gather (indirect DMA) + PE-transpose -> xT; input projection
      x @ WihT (+bias) for both directions -> xprojT staged in DRAM.
  P2: LSTM recurrence, fwd and bwd interleaved as two independent chains.
      Gates computed transposed ([u, b] layout, u on partitions) so the
      elementwise/activation work is tiny (batch=8 on the free dim).
  P3: CRF Viterbi forward: feats matmul on PE; dp update via broadcast-add +
      segmented max using tensor_tensor_scan with reset columns.
  P4: terminal scores; P5: backtrack via one-hot matmul gather of trans
      columns + max/max_index.

All per-core programs are identical (SPMD); no cross-core communication.
"""

import numpy as np
from contextlib import ExitStack

import concourse.bass as bass
import concourse.bacc as bacc
import concourse.mybir as mybir
import concourse.tile as tile
from concourse.alu_op_type import AluOpType
from concourse.masks import make_identity

F32 = mybir.dt.float32
I32 = mybir.dt.int32
U32 = mybir.dt.uint32
AF = mybir.ActivationFunctionType

# Problem constants
B, T_FULL, V_FULL, E, HS, NL = 64, 512, 50000, 256, 512, 34
H = 256
BL = 8          # batch per core
NCORES = 8
START_IDX, STOP_IDX = 32, 33
NEG = -10000.0
BIGNEG = -1.0e30
KP = NL + 1     # padded k-stride for the scan (35)


def build_nc(T=T_FULL, V=V_FULL):
    """Build the per-core bass program (SPMD: identical on all cores)."""
    nc = bacc.Bacc("TRN2", target_bir_lowering=False)
    T8 = T * BL

    # ---- DRAM I/O ----
    embed_d = nc.dram_tensor("embed", [V, E], F32, kind="ExternalInput")
    toks_d = nc.dram_tensor("toks", [T8, 1], I32, kind="ExternalInput")
    whhT_d = [nc.dram_tensor(f"whhT_{d}", [128, 2048], F32, kind="ExternalInput") for d in range(2)]
    wihT_d = [nc.dram_tensor(f"wihT_{d}", [128, 2048], F32, kind="ExternalInput") for d in range(2)]
    biasT_d = [nc.dram_tensor(f"biasT_{d}", [128, 8], F32, kind="ExternalInput") for d in range(2)]
    h0T_d = [nc.dram_tensor(f"h0T_{d}", [128, 16], F32, kind="ExternalInput") for d in range(2)]
    c0T_d = [nc.dram_tensor(f"c0T_{d}", [128, 16], F32, kind="ExternalInput") for d in range(2)]
    w_outT_d = nc.dram_tensor("w_outT", [128, 4 * NL], F32, kind="ExternalInput")
    ttrep_d = nc.dram_tensor("transT_rep", [BL, NL * NL], F32, kind="ExternalInput")
    ttplain_d = nc.dram_tensor("transT_plain", [NL, NL], F32, kind="ExternalInput")
    stoprep_d = nc.dram_tensor("stoprep", [BL, NL], F32, kind="ExternalInput")
    rmask_d = nc.dram_tensor("resetmask", [BL, NL * KP], F32, kind="ExternalInput")
    iota_d = nc.dram_tensor("iota34", [BL, NL], F32, kind="ExternalInput")
    best_d = nc.dram_tensor("best", [BL, 1], F32, kind="ExternalOutput")
    path_d = nc.dram_tensor("path", [BL, T], I32, kind="ExternalOutput")

    with tile.TileContext(nc) as tc, ExitStack() as ctx:
        cp = ctx.enter_context(tc.tile_pool(name="consts", bufs=1))
        dpool = ctx.enter_context(tc.tile_pool(name="dram", bufs=1, space="DRAM"))

        whh_raw = [cp.tile_from(whhT_d[d][:], name=f"whhr{d}") for d in range(2)]
        whh = []
        for d in range(2):
            w2 = cp.tile([128, 2048], F32, tag=f"whh{d}", name=f"whh{d}")
            nc.vector.tensor_copy(w2, whh_raw[d])
            whh.append(w2)
        xpd = [dpool.tile([128, T * 64], F32, tag=f"xpd{d}", name=f"xpd{d}") for d in range(2)]

        # ================= Phase 1: embedding + input projection =================
        with tc.tile_pool(name="p1c", bufs=1) as p1c, \
             tc.tile_pool(name="p1", bufs=4) as p1, \
             tc.tile_pool(name="p1tr", bufs=4, space="PSUM") as p1tr, \
             tc.tile_pool(name="p1mm", bufs=2, space="PSUM") as p1mm:
            ident_raw = p1c.tile([128, 128], F32, tag="identr")
            make_identity(nc, ident_raw)
            ident = p1c.tile([128, 128], F32, tag="ident")
            nc.vector.tensor_copy(ident, ident_raw)
            wih_raw = [p1c.tile_from(wihT_d[d][:], name=f"wihr{d}") for d in range(2)]
            wih = []
            for d in range(2):
                w2 = p1c.tile([128, 2048], F32, tag=f"wih{d}", name=f"wih{d}")
                nc.vector.tensor_copy(w2, wih_raw[d])
                wih.append(w2)
            biasT_raw = [p1c.tile_from(biasT_d[d][:], name=f"biasTr{d}") for d in range(2)]
            biasT = []
            for d in range(2):
                b2 = p1c.tile([128, 8], F32, tag=f"biasTs{d}", name=f"biasTs{d}")
                nc.vector.tensor_copy(b2, biasT_raw[d])
                biasT.append(b2)
            xT = p1c.tile([128, 2 * T8], F32, tag="xT")

            ntile = T8 // 128
            for m in range(ntile):
                tokt = p1.tile([128, 1], I32, tag="tok")
                nc.sync.dma_start(out=tokt, in_=toks_d[m * 128:(m + 1) * 128, :])
                xr = p1.tile([128, E], F32, tag="xr")
                nc.gpsimd.indirect_dma_start(
                    out=xr, out_offset=None, in_=embed_d[:],
                    in_offset=bass.IndirectOffsetOnAxis(ap=tokt[:, 0:1], axis=0))
                xr2 = p1.tile([128, E], F32, tag="xr2")
                nc.vector.tensor_copy(xr2, xr)
                for k2 in range(2):
                    pst = p1tr.tile([128, 128], F32, tag="tr")
                    nc.tensor.transpose(pst, xr2[:, k2 * 128:(k2 + 1) * 128], ident)
                    nc.vector.tensor_copy(
                        xT[:, k2 * T8 + m * 128: k2 * T8 + (m + 1) * 128], pst)

            CHW = min(512, T8)          # xproj chunk width in (t, b) columns
            for d in range(2):
                for tch in range(T8 // CHW):
                    for m in range(8):
                        ps = p1mm.tile([128, CHW], F32, tag="xps")
                        for k2 in range(2):
                            nc.tensor.matmul(
                                ps,
                                wih[d][:, k2 * 1024 + m * 128: k2 * 1024 + (m + 1) * 128],
                                xT[:, k2 * T8 + tch * CHW: k2 * T8 + (tch + 1) * CHW],
                                start=(k2 == 0), stop=(k2 == 1))
                        st = p1.tile([128, CHW], F32, tag="xst")
                        nc.vector.tensor_scalar(
                            st, ps, biasT[d][:, m:m + 1], None, op0=AluOpType.add)
                        dst = xpd[d].rearrange("p (t mb) -> p t mb", mb=64)[
                            :, tch * (CHW // 8):(tch + 1) * (CHW // 8), m * 8:(m + 1) * 8]
                        nc.sync.dma_start(out=dst, in_=st.rearrange("p (t b) -> p t b", b=8))

        # ================= Phase 2: BiLSTM recurrence =================
        p2c = ctx.enter_context(tc.tile_pool(name="p2c", bufs=1))
        hbuf = [p2c.tile([128, T * 16], F32, tag=f"hb{d}", name=f"hb{d}") for d in range(2)]
        h0T_raw = [p2c.tile_from(h0T_d[d][:], name=f"h0Tr{d}") for d in range(2)]
        h0T = []
        for d in range(2):
            t2 = p2c.tile([128, 16], F32, tag=f"h0Ts{d}", name=f"h0Ts{d}")
            nc.vector.tensor_copy(t2, h0T_raw[d])
            h0T.append(t2)
        c0T = [p2c.tile_from(c0T_d[d][:], name=f"c0Ts{d}") for d in range(2)]

        with tc.tile_pool(name="p2", bufs=3) as p2, \
             tc.tile_pool(name="p2x", bufs=6) as p2x, \
             tc.tile_pool(name="p2cc", bufs=3) as p2cc, \
             tc.tile_pool(name="p2ps", bufs=2, space="PSUM") as p2ps:
            hprev = [h0T[0], h0T[1]]
            cprev = [c0T[0], c0T[1]]
            for t in range(T):
                for d in range(2):
                    td = t if d == 0 else T - 1 - t
                    xp = p2x.tile([128, 64], F32, tag=f"xp{d}")
                    nc.sync.dma_start(out=xp, in_=xpd[d][:, td * 64:(td + 1) * 64])
                    psg = p2ps.tile([128, 64], F32, tag=f"psg{d}")
                    for m in range(8):
                        for k2 in range(2):
                            nc.tensor.matmul(
                                psg[:, m * 8:(m + 1) * 8],
                                whh[d][:, k2 * 1024 + m * 128: k2 * 1024 + (m + 1) * 128],
                                hprev[d][:, k2 * 8:(k2 + 1) * 8],
                                start=(k2 == 0), stop=(k2 == 1))
                    ga = p2.tile([128, 64], F32, tag=f"ga{d}")
                    nc.vector.tensor_add(ga, psg, xp)
                    # gate order is [i, f, o, g] (host permutes weight rows)
                    nc.scalar.activation(ga[:, 0:48], ga[:, 0:48], AF.Sigmoid)
                    nc.scalar.activation(ga[:, 48:64], ga[:, 48:64], AF.Tanh)
                    ig = p2.tile([128, 16], F32, tag=f"ig{d}")
                    nc.vector.tensor_mul(ig, ga[:, 0:16], ga[:, 48:64])
                    fc = p2.tile([128, 16], F32, tag=f"fc{d}")
                    nc.vector.tensor_mul(fc, ga[:, 16:32], cprev[d])
                    cn = p2cc.tile([128, 16], F32, tag=f"c{d}")
                    nc.vector.tensor_add(cn, ig, fc)
                    tct = p2.tile([128, 16], F32, tag=f"tc{d}")
                    nc.scalar.activation(tct, cn, AF.Tanh)
                    hslot = hbuf[d][:, td * 16:(td + 1) * 16]
                    nc.vector.tensor_mul(hslot, ga[:, 32:48], tct)
                    hprev[d] = hslot
                    cprev[d] = cn

        # ================= Phase 3: CRF forward =================
        p3c = ctx.enter_context(tc.tile_pool(name="p3c", bufs=1))
        dpbuf = p3c.tile([BL, (T + 1) * NL], F32, tag="dpbuf")
        w_out_raw = p3c.tile_from(w_outT_d[:], name="w_out_raw")
        w_out_sb = p3c.tile([128, 4 * NL], F32, tag="w_out_sb", name="w_out_sb")
        nc.vector.tensor_copy(w_out_sb, w_out_raw)
        ttrep = p3c.tile_from(ttrep_d[:])
        rmask = p3c.tile_from(rmask_d[:])
        tmp = p3c.tile([BL, NL * KP], F32, tag="tmp")
        scano = p3c.tile([BL, NL * KP], F32, tag="scano")
        nc.vector.memset(tmp, BIGNEG)
        nc.vector.memset(dpbuf[:, 0:NL], NEG)
        nc.vector.memset(dpbuf[:, START_IDX:START_IDX + 1], 0.0)

        tmp3 = tmp.rearrange("p (j k) -> p j k", k=KP)[:, :, 0:NL]
        ttrep3 = ttrep.rearrange("p (j k) -> p j k", k=NL)
        segmax = scano.rearrange("p (j k) -> p j k", k=KP)[:, :, NL - 1:NL].squeeze(-1)
        with tc.tile_pool(name="p3ps", bufs=2, space="PSUM") as p3ps:
            for t in range(T):
                psf = p3ps.tile([BL, NL], F32, tag="psf")
                for kk in range(4):
                    hb = hbuf[0] if kk < 2 else hbuf[1]
                    k2 = kk % 2
                    nc.tensor.matmul(
                        psf, hb[:, t * 16 + k2 * 8: t * 16 + k2 * 8 + 8],
                        w_out_sb[:, kk * NL:(kk + 1) * NL],
                        start=(kk == 0), stop=(kk == 3))
                dp_t = dpbuf[:, t * NL:(t + 1) * NL]
                nc.vector.tensor_tensor(
                    tmp3, dp_t.unsqueeze(1).broadcast_to([BL, NL, NL]), ttrep3,
                    op=AluOpType.add)
                nc.vector.tensor_tensor_scan(
                    scano, tmp, rmask, initial=BIGNEG,
                    op0=AluOpType.max, op1=AluOpType.min)
                nc.vector.scalar_tensor_tensor(
                    dpbuf[:, (t + 1) * NL:(t + 2) * NL], segmax, 0.0, psf,
                    op0=AluOpType.add, op1=AluOpType.add)

        # ================= Phase 4/5: terminal + backtrack =================
        stopr = p3c.tile_from(stoprep_d[:])
        iota_sb = p3c.tile_from(iota_d[:])
        ttplain_raw = p3c.tile_from(ttplain_d[:], name="ttplain_raw")
        ttplain = p3c.tile([NL, NL], F32, tag="ttplain", name="ttplain")
        nc.vector.tensor_copy(ttplain, ttplain_raw)
        pathb = p3c.tile([BL, T], I32, tag="pathb")
        bestv = p3c.tile([BL, 1], F32, tag="bestv")
        jcur = p3c.tile([BL, 1], F32, tag="jcur")
        tt2 = p3c.tile([BL, NL], F32, tag="tt2")
        ident8_raw = p3c.tile([8, 8], F32, tag="ident8r")
        make_identity(nc, ident8_raw)
        ident8 = p3c.tile([8, 8], F32, tag="ident8")
        nc.vector.tensor_copy(ident8, ident8_raw)

        with tc.tile_pool(name="p5", bufs=3) as p5, \
             tc.tile_pool(name="p5ps", bufs=2, space="PSUM") as p5ps:
            nc.vector.tensor_add(tt2, dpbuf[:, T * NL:(T + 1) * NL], stopr)
            nc.vector.reduce_max(bestv, tt2, axis=mybir.AxisListType.X)
            mx8 = p5.tile([BL, 8], F32, tag="mx8")
            ix8 = p5.tile([BL, 8], U32, tag="ix8")
            nc.vector.max(out=mx8, in_=tt2)
            nc.vector.max_index(out=ix8, in_max=mx8, in_values=tt2)
            nc.vector.tensor_copy(jcur, ix8[:, 0:1])
            nc.vector.tensor_copy(pathb[:, T - 1:T], ix8[:, 0:1])
            for t in range(T - 1, 0, -1):
                ohb = p5.tile([BL, NL], F32, tag="ohb")
                nc.vector.tensor_scalar(
                    ohb, iota_sb, jcur[:, 0:1], None, op0=AluOpType.is_equal)
                ohT_ps = p5ps.tile([NL, 8], F32, tag="ohT")
                nc.tensor.transpose(ohT_ps, ohb, ident8)
                ohT = p5.tile([NL, 8], F32, tag="ohTs")
                nc.vector.tensor_copy(ohT, ohT_ps)
                tcol = p5ps.tile([BL, NL], F32, tag="tcol")
                nc.tensor.matmul(tcol, ohT, ttplain, start=True, stop=True)
                sc = p5.tile([BL, NL], F32, tag="sc")
                nc.vector.tensor_add(sc, dpbuf[:, t * NL:(t + 1) * NL], tcol)
                mxb = p5.tile([BL, 8], F32, tag="mxb")
                ixb = p5.tile([BL, 8], U32, tag="ixb")
                nc.vector.max(out=mxb, in_=sc)
                nc.vector.max_index(out=ixb, in_max=mxb, in_values=sc)
                nc.vector.tensor_copy(pathb[:, t - 1:t], ixb[:, 0:1])
                nc.vector.tensor_copy(jcur, ixb[:, 0:1])

        nc.sync.dma_start(out=best_d[:], in_=bestv)
        nc.sync.dma_start(out=path_d[:], in_=pathb)
    return nc


# ======================= host-side marshaling =======================

def _permute_gates(W):
    """PyTorch gate order [i, f, g, o] -> kernel order [i, f, o, g] (rows)."""
    W4 = W.reshape(4, H, *W.shape[1:])
    return np.concatenate([W4[0], W4[1], W4[3], W4[2]], axis=0)


def _wT_layout(W):
    """W [1024, 256] -> [128, 2048]: out[p, k2*1024+m*128+c] = W[m*128+c, k2*128+p]."""
    Wr = W.reshape(8, 128, 2, 128)                      # m, c, k2, p
    return np.ascontiguousarray(Wr.transpose(3, 2, 0, 1).reshape(128, 2048)).astype(np.float32)


def _h0T_layout(h):
    """h [8, 256] -> [128, 16]: out[p, k2*8+b] = h[b, k2*128+p]."""
    return np.ascontiguousarray(
        h.reshape(BL, 2, 128).transpose(2, 1, 0).reshape(128, 16)).astype(np.float32)


def marshal(inputs, T=T_FULL):
    f32 = np.float32
    emb = np.asarray(inputs["embed"], dtype=f32)
    shared = {"embed": emb}
    for d, sfx in ((0, "f"), (1, "b")):
        Wih = _permute_gates(np.asarray(inputs[f"Wih_{sfx}"], dtype=f32))
        Whh = _permute_gates(np.asarray(inputs[f"Whh_{sfx}"], dtype=f32))
        bias = _permute_gates(
            np.asarray(inputs[f"bih_{sfx}"], dtype=f32)
            + np.asarray(inputs[f"bhh_{sfx}"], dtype=f32))
        shared[f"wihT_{d}"] = _wT_layout(Wih)
        # Whh is [1024, 256]: contraction dim H=256 -> same layout helper
        shared[f"whhT_{d}"] = _wT_layout(Whh)
        shared[f"biasT_{d}"] = np.ascontiguousarray(bias.reshape(8, 128).T).astype(f32)
    W_out = np.asarray(inputs["W_out"], dtype=f32)      # [34, 512]
    shared["w_outT"] = np.ascontiguousarray(
        W_out.reshape(NL, 4, 128).transpose(2, 1, 0).reshape(128, 4 * NL)).astype(f32)
    trans = np.asarray(inputs["trans"], dtype=f32)
    b_out = np.asarray(inputs["b_out"], dtype=f32)
    shared["transT_rep"] = np.tile(
        (trans.T + b_out[:, None]).reshape(1, NL * NL), (BL, 1)).astype(f32)
    shared["transT_plain"] = np.ascontiguousarray(trans.T).astype(f32)
    shared["stoprep"] = np.tile(trans[:, STOP_IDX].reshape(1, NL), (BL, 1)).astype(f32)
    rm = np.full((NL, KP), 1.0e30, f32)
    rm[:, NL] = BIGNEG
    shared["resetmask"] = np.tile(rm.reshape(1, NL * KP), (BL, 1)).astype(f32)
    shared["iota34"] = np.tile(np.arange(NL, dtype=f32).reshape(1, NL), (BL, 1))

    sents = np.asarray(inputs["sentences"]).astype(np.int32)
    h0 = np.asarray(inputs["h0"], dtype=f32)
    c0 = np.asarray(inputs["c0"], dtype=f32)
    ncores = sents.shape[0] // BL
    in_maps = []
    for c in range(ncores):
        m = dict(shared)
        s = sents[c * BL:(c + 1) * BL]                  # [8, T]
        m["toks"] = np.ascontiguousarray(s.T.reshape(T * BL, 1)).astype(np.int32)
        for d in range(2):
            m[f"h0T_{d}"] = _h0T_layout(h0[d, c * BL:(c + 1) * BL])
            m[f"c0T_{d}"] = _h0T_layout(c0[d, c * BL:(c + 1) * BL])
        in_maps.append(m)
    return in_maps


_NC_CACHE = {}
LAST_RESULTS = None


def kernel(**inputs):
    global LAST_RESULTS
    from concourse.bass_utils import run_bass_kernel_spmd
    sents = np.asarray(inputs["sentences"])
    T = sents.shape[1]
    V = np.asarray(inputs["embed"]).shape[0]
    key = (T, V)
    if key not in _NC_CACHE:
        nc = build_nc(T=T, V=V)
        if not nc.is_finalized():
            nc.finalize()
        _NC_CACHE[key] = nc
    nc = _NC_CACHE[key]
    in_maps = marshal(inputs, T=T)
    res = run_bass_kernel_spmd(nc, in_maps, core_ids=list(range(len(in_maps))))
    LAST_RESULTS = res
    best = np.concatenate([r["best"].reshape(BL) for r in res.results])
    path = np.concatenate([r["path"].reshape(BL, T) for r in res.results], axis=0)
    return best.astype(np.float32), path.astype(np.int32)
